# revision 2
# baseline (speedup 1.0000x reference)
"""Trainium2 Bass kernel for nn_EntityClassify (2-layer R-GCN on 8 NeuronCores).

Math (matches reference):
  h1  = relu(bias1 + sum_r S_r @ embed)          S_r = right-normalized adjacency
  out = bias2 + sum_r S_r @ (h1 @ W_r)

Distribution: destination nodes sharded across 8 cores; embed + weights
replicated. Aggregation is race-free one-hot matmul accumulation in PSUM
(HW dma_scatter_add loses colliding updates across its 16 parallel engines,
so no scatter-add is used):

  L1 per 128-edge tile: dma_gather embed rows (fp16) -> M[e,s] =
    w_e * (dst_local==s) built by is_equal against an iota row ->
    PSUM[h,128] += msg^T M; per (chunk,block) run added into an SBUF
    accumulator [128h, shard]; epilogue relu(bias1+acc) -> transpose ->
    fp16 h1 shard.
  AllGather h1 (fp16).
  L2 per tile: transposed dma_gather of h1 (gives msg^T [h,e]) -> 4 matmuls
    with W_r into one PSUM [e,4*64] -> ACT scaled-copy (scale=w_e, cast fp16)
    -> 4 agg matmuls with rel-blocked one-hot M512 (dl4 = rel*128+dst_local)
    into PSUM [64,128] per (chunk,block) -> SBUF accumulator [64o, shard] ->
    epilogue +bias2, transpose, store.

Host precomputes per-core edge schedules; per-(chunk,block) capacities are
maxed over cores so the single SPMD program fits every core; pads use
(idx=0, w=0, dl=999) no-op edges.
"""

import os
import sys

import numpy as np

sys.path.insert(0, "/opt/trn_rl_repo")

NCORES = 8
NCHUNKS = 4
BATCH = 2048  # max indices per dma_gather call (needs single_packet=False on HW)
P = 128

last_results = None  # BassKernelResults of the most recent hw run


def _round_up(x, m):
    return (x + m - 1) // m * m


def _wrap16(idx, n):
    """SWDGE index layout: position j -> [j%16, j//16]; 16 rows replicated x8."""
    a = idx.reshape(n // 16, 16).T.astype(np.int16)
    return np.tile(a, (8, 1))


def _build_schedule(src, dst, wts, dl, chunk, shard, nblk):
    """Sort edges by (chunk, block); pad per group to x128 with no-op edges.

    Returns per-core packed streams plus the shared group capacities.
    dl: per-edge one-hot column (dst_local for L1, rel*128+dst_local for L2).
    """
    core = dst // shard
    block = (dst % shard) // P
    group = (src // chunk) * nblk + block
    ngroups = NCHUNKS * nblk

    per_core = []
    counts = np.zeros((NCORES, ngroups), np.int64)
    for k in range(NCORES):
        m = core == k
        g = group[m]
        order = np.argsort(g, kind="stable")
        per_core.append((g[order], src[m][order], wts[m][order], dl[m][order]))
        counts[k] = np.bincount(g, minlength=ngroups)
    caps = _round_up(counts.max(axis=0), P)
    offs = np.concatenate([[0], np.cumsum(caps)])
    tot = int(caps.sum())

    gidx = np.zeros((NCORES, tot), np.int16)
    warr = np.zeros((NCORES, tot), np.float16)
    dlarr = np.full((NCORES, tot), 999.0, np.float16)  # pads never match iota
    for k in range(NCORES):
        g, s, w, d = per_core[k]
        gs = np.searchsorted(g, np.arange(ngroups))
        ge = np.searchsorted(g, np.arange(ngroups), side="right")
        for gi in range(ngroups):
            a, b = gs[gi], ge[gi]
            o = offs[gi]
            n = b - a
            gidx[k, o : o + n] = (s[a:b] % chunk).astype(np.int16)
            warr[k, o : o + n] = w[a:b].astype(np.float16)
            dlarr[k, o : o + n] = d[a:b].astype(np.float16)
    return gidx, warr, dlarr, caps, offs


def _make_calls(caps, nblk):
    """Chunk-pure gather-call windows (<=BATCH, x128) over the packed stream,
    plus per-tile group bookkeeping: (group, is_first, is_last)."""
    offs = np.concatenate([[0], np.cumsum(caps)])
    calls = []  # (chunk, stream_off, n)
    tiles = []  # per global tile: (group, first, last)
    for c in range(NCHUNKS):
        lo = int(offs[c * nblk])
        hi = int(offs[(c + 1) * nblk])
        o = lo
        while o < hi:
            n = min(BATCH, hi - o)
            calls.append((c, o, n))
            o += n
    ntiles_total = int(caps.sum()) // P
    tile_group = np.zeros(ntiles_total, np.int64)
    for gi, cap in enumerate(caps):
        t0 = int(offs[gi]) // P
        for t in range(int(cap) // P):
            tile_group[t0 + t] = gi
    for t in range(ntiles_total):
        g = tile_group[t]
        first = t == 0 or tile_group[t - 1] != g
        last = t == ntiles_total - 1 or tile_group[t + 1] != g
        tiles.append((int(g), first, last))
    return calls, tiles


def _host_schedules(embed, weight, bias1, bias2, edge_src, edge_dst):
    N, H = embed.shape
    R, _, O = weight.shape
    E = edge_src.shape[1]
    shard = _round_up((N + NCORES - 1) // NCORES, P)
    npad = shard * NCORES
    chunk = npad // NCHUNKS
    nblk = shard // P
    assert chunk < 32768 and shard < 32768

    es = edge_src.astype(np.int64).reshape(R, E)
    ed = edge_dst.astype(np.int64).reshape(R, E)
    deg = np.zeros((R, N), np.float32)
    for r in range(R):
        deg[r] = np.bincount(ed[r], minlength=N)
    dinv = 1.0 / np.maximum(deg, 1.0)

    src = es.reshape(-1)
    dst = ed.reshape(-1)
    rel = np.repeat(np.arange(R), E)
    w = dinv[rel, dst]
    dl1 = ((dst % shard) % P).astype(np.float64)
    dl2 = (rel * P + (dst % shard) % P).astype(np.float64)

    g1, w1, d1, caps1, _ = _build_schedule(src, dst, w, dl1, chunk, shard, nblk)
    g2, w2, d2, caps2, _ = _build_schedule(src, dst, w, dl2, chunk, shard, nblk)
    calls1, tiles1 = _make_calls(caps1, nblk)
    calls2, tiles2 = _make_calls(caps2, nblk)
    tot1, tot2 = int(caps1.sum()), int(caps2.sum())

    iota = np.zeros((P, 4 * P), np.float16)
    iota[:] = np.arange(4 * P, dtype=np.float16)[None, :]
    ident = np.eye(P, dtype=np.float32)

    consts = dict(
        N=N, H=H, R=R, O=O, shard=shard, npad=npad, chunk=chunk, nblk=nblk,
        tot1=tot1, tot2=tot2, calls1=calls1, calls2=calls2,
        tiles1=tiles1, tiles2=tiles2,
    )
    embed_pad = np.zeros((npad, H), np.float16)
    embed_pad[:N] = embed.astype(np.float16)
    in_maps = []
    for k in range(NCORES):
        in_maps.append(
            dict(
                embed=embed_pad,
                wrel=np.ascontiguousarray(weight.astype(np.float16)),
                b1c=bias1.astype(np.float32).reshape(P, 1),
                b2c=bias2.astype(np.float32).reshape(O, 1),
                iota=iota,
                ident=ident,
                g1=_wrap16(g1[k], tot1),
                w1=w1[k].reshape(tot1 // P, P).T.copy(),
                d1=d1[k].reshape(tot1 // P, P).T.copy(),
                g2=_wrap16(g2[k], tot2),
                w2=w2[k].reshape(tot2 // P, P).T.astype(np.float32).copy(),
                d2=d2[k].reshape(tot2 // P, P).T.copy(),
            )
        )
    return consts, in_maps


def _simulate_numpy(consts, in_maps):
    """Numpy model of exactly what the device program computes."""
    shard, chunk, H, O, R, nblk = (
        consts["shard"], consts["chunk"], consts["H"], consts["O"],
        consts["R"], consts["nblk"],
    )

    def unwrap(a, tot):
        return a[:16].T.reshape(-1)[:tot].astype(np.int64)

    h1_all = []
    for k in range(NCORES):
        m = in_maps[k]
        gi = unwrap(m["g1"], consts["tot1"])
        w = m["w1"].T.reshape(-1)
        dl = m["d1"].T.reshape(-1)
        acc = np.zeros((H, shard), np.float32)
        for t, (g, first, last) in enumerate(consts["tiles1"]):
            c, b = g // nblk, g % nblk
            sl = slice(t * P, (t + 1) * P)
            msg = m["embed"][c * chunk + gi[sl]].astype(np.float32)  # [e, h]
            M = (dl[sl][:, None] == np.arange(P)).astype(np.float32)
            M *= w[sl].astype(np.float32)[:, None]
            acc[:, b * P : (b + 1) * P] += msg.T @ M
        h1 = np.maximum(acc + m["b1c"], 0).T.astype(np.float16)  # [shard, H]
        h1_all.append(h1)
    h1f = np.concatenate(h1_all, 0)

    outs = []
    for k in range(NCORES):
        m = in_maps[k]
        gi = unwrap(m["g2"], consts["tot2"])
        w = m["w2"].T.reshape(-1)
        dl = m["d2"].T.reshape(-1)
        acc = np.zeros((O, shard), np.float32)
        wr = m["wrel"].astype(np.float32)
        for t, (g, first, last) in enumerate(consts["tiles2"]):
            c, b = g // nblk, g % nblk
            sl = slice(t * P, (t + 1) * P)
            msg = h1f[c * chunk + gi[sl]].astype(np.float32)  # [e, h]
            xw = np.stack([msg @ wr[r] for r in range(R)], 1)  # [e, R, O]
            xw = (xw * w[sl].astype(np.float32)[:, None, None]).astype(np.float16)
            M = (dl[sl][:, None] == np.arange(R * P)).astype(np.float16)
            for r in range(R):
                acc[:, b * P : (b + 1) * P] += (
                    xw[:, r, :].astype(np.float32).T
                    @ M[:, r * P : (r + 1) * P].astype(np.float32)
                )
        outs.append((acc + m["b2c"]).T)  # [shard, O]
    return np.concatenate(outs, 0)[: consts["N"]]


def _build_program(consts, finalize):
    import concourse.bacc as bacc
    import concourse.mybir as mybir
    import concourse.tile as tile
    from concourse import library_config

    f32 = mybir.dt.float32
    f16 = mybir.dt.float16
    i16 = mybir.dt.int16
    AF = mybir.ActivationFunctionType
    H, O, R = consts["H"], consts["O"], consts["R"]
    shard, npad, chunk, nblk = (
        consts["shard"], consts["npad"], consts["chunk"], consts["nblk"],
    )
    tot1, tot2 = consts["tot1"], consts["tot2"]

    nc = bacc.Bacc("TRN2")
    embed = nc.declare_dram_parameter("embed", [npad, H], f16, isOutput=False)
    wrel = nc.declare_dram_parameter("wrel", [R, H, O], f16, isOutput=False)
    b1c = nc.declare_dram_parameter("b1c", [P, 1], f32, isOutput=False)
    b2c = nc.declare_dram_parameter("b2c", [O, 1], f32, isOutput=False)
    iota = nc.declare_dram_parameter("iota", [P, R * P], f16, isOutput=False)
    ident = nc.declare_dram_parameter("ident", [P, P], f32, isOutput=False)
    g1 = nc.declare_dram_parameter("g1", [P, tot1 // 16], i16, isOutput=False)
    w1 = nc.declare_dram_parameter("w1", [P, tot1 // P], f16, isOutput=False)
    d1 = nc.declare_dram_parameter("d1", [P, tot1 // P], f16, isOutput=False)
    g2 = nc.declare_dram_parameter("g2", [P, tot2 // 16], i16, isOutput=False)
    w2 = nc.declare_dram_parameter("w2", [P, tot2 // P], f32, isOutput=False)
    d2 = nc.declare_dram_parameter("d2", [P, tot2 // P], f16, isOutput=False)
    out = nc.declare_dram_parameter("out", [shard, O], f32, isOutput=True)

    h1l = nc.dram_tensor("h1l", [shard, H], f16)
    h1f = nc.dram_tensor("h1f", [npad, H], f16, addr_space="Shared")

    with tile.TileContext(nc) as tc:
        with (
            tc.tile_pool(name="const", bufs=1) as cpool,
            tc.tile_pool(name="acc", bufs=1) as apool,
            tc.tile_pool(name="idx", bufs=4) as ipool,
            tc.tile_pool(name="stage", bufs=4) as spool,
            tc.tile_pool(name="mbuf", bufs=3) as mpool,
            tc.tile_pool(name="xw", bufs=3) as xpool,
            tc.tile_pool(name="ep", bufs=3) as epool,
            tc.tile_pool(name="agg_ps", bufs=3, space="PSUM") as qagg,
            tc.tile_pool(name="xw_ps", bufs=2, space="PSUM") as qxw,
            tc.tile_pool(name="tr_ps", bufs=2, space="PSUM") as qtr,
        ):
            nc.gpsimd.load_library(library_config.mlp)

            _regs = {}

            def nreg(n):
                if n not in _regs:
                    r = nc.gpsimd.alloc_register(name=f"nidx{n}")
                    nc.gpsimd.reg_mov(r, n)
                    _regs[n] = r
                return _regs[n]

            b1t = cpool.tile([P, 1], f32)
            nc.sync.dma_start(out=b1t[:], in_=b1c[:, :])
            b2t = cpool.tile([O, 1], f32)
            nc.sync.dma_start(out=b2t[:], in_=b2c[:, :])
            iot = cpool.tile([P, R * P], f16)
            nc.sync.dma_start(out=iot[:], in_=iota[:, :])
            idt = cpool.tile([P, P], f32)
            nc.sync.dma_start(out=idt[:], in_=ident[:, :])
            wts = []
            for r in range(R):
                wt = cpool.tile([P, O], f16, tag=f"wrel{r}")
                nc.sync.dma_start(out=wt[:], in_=wrel[r, :, :])
                wts.append(wt)

            acc1 = apool.tile([P, shard], f32)
            nc.any.memset(acc1[:], 0.0)
            acc2 = apool.tile([O, shard], f32)
            nc.any.memset(acc2[:], 0.0)

            # ---------------- layer 1 ----------------
            psums = {}
            t_base = 0
            for c, off, n in consts["calls1"]:
                k = n // P
                git = ipool.tile([P, BATCH // 16], i16, tag="g1t")
                nc.sync.dma_start(out=git[:, : n // 16], in_=g1[:, off // 16 : (off + n) // 16])
                wtt = ipool.tile([P, BATCH // P], f16, tag="w1t")
                nc.sync.dma_start(out=wtt[:, :k], in_=w1[:, off // P : off // P + k])
                dlt = ipool.tile([P, BATCH // P], f16, tag="d1t")
                nc.sync.dma_start(out=dlt[:, :k], in_=d1[:, off // P : off // P + k])
                st = spool.tile([P, BATCH // P, H], f16, tag="st1")
                nc.gpsimd.dma_gather(
                    out_ap=st[:, :k, :],
                    in_ap=embed[c * chunk : (c + 1) * chunk, :],
                    idxs_ap=git[:, : n // 16],
                    num_idxs=n,
                    num_idxs_reg=nreg(n),
                    elem_size=H,
                    single_packet=False,
                )
                mb = mpool.tile([P, BATCH // P, P], f16, tag="m1")
                nc.any.tensor_tensor(
                    mb[:, :k, :],
                    dlt[:, :k, None].to_broadcast([P, k, P]),
                    iot[:, None, :P].to_broadcast([P, k, P]),
                    op=mybir.AluOpType.is_equal,
                )
                nc.any.tensor_tensor(
                    mb[:, :k, :],
                    mb[:, :k, :],
                    wtt[:, :k, None].to_broadcast([P, k, P]),
                    op=mybir.AluOpType.mult,
                )
                for t in range(k):
                    g, first, last = consts["tiles1"][t_base + t]
                    if first:
                        psums[g] = qagg.tile([P, P], f32, tag="agg", name=f"agg1_{g}")
                    nc.tensor.matmul(
                        psums[g][:],
                        lhsT=st[:, t, :],
                        rhs=mb[:, t, :],
                        start=first,
                        stop=last,
                    )
                    if last:
                        b = g % nblk
                        nc.any.tensor_add(
                            acc1[:, b * P : (b + 1) * P],
                            acc1[:, b * P : (b + 1) * P],
                            psums[g][:],
                        )
                        del psums[g]
                t_base += k

            # ---- h1 = relu(acc1 + b1), transpose to [slot, h], fp16
            for b in range(nblk):
                sl = slice(b * P, (b + 1) * P)
                hb = epool.tile([P, P], f32, tag="ep_h")
                nc.scalar.activation(hb[:], acc1[:, sl], AF.Relu, bias=b1t[:])
                tp = qtr.tile([P, P], f32, tag="ep_t")
                nc.tensor.transpose(out=tp[:], in_=hb[:], identity=idt[:])
                hf = epool.tile([P, P], f16, tag="ep_f")
                nc.scalar.activation(hf[:], tp[:], AF.Copy)
                nc.sync.dma_start(out=h1l[sl, :], in_=hf[:])

            # ---- all-gather h1
            nc.gpsimd.collective_compute(
                "AllGather",
                mybir.AluOpType.bypass,
                replica_groups=[list(range(NCORES))],
                ins=[h1l[:, :]],
                outs=[h1f[:, :]],
            )

            # ---------------- layer 2 ----------------
            psums = {}
            t_base = 0
            for c, off, n in consts["calls2"]:
                k = n // P
                git = ipool.tile([P, BATCH // 16], i16, tag="g2t")
                nc.sync.dma_start(out=git[:, : n // 16], in_=g2[:, off // 16 : (off + n) // 16])
                wtt = ipool.tile([P, BATCH // P], f32, tag="w2t")
                nc.sync.dma_start(out=wtt[:, :k], in_=w2[:, off // P : off // P + k])
                dlt = ipool.tile([P, BATCH // P], f16, tag="d2t")
                nc.sync.dma_start(out=dlt[:, :k], in_=d2[:, off // P : off // P + k])
                mst = spool.tile([P, 1, BATCH], f16, tag="st2")
                nc.gpsimd.dma_gather(
                    out_ap=mst[:, :, :n],
                    in_ap=h1f[c * chunk : (c + 1) * chunk, :],
                    idxs_ap=git[:, : n // 16],
                    num_idxs=n,
                    num_idxs_reg=nreg(n),
                    elem_size=H,
                    transpose=True,
                    single_packet=False,
                )
                mb = mpool.tile([P, BATCH // P, R * P], f16, tag="m2")
                nc.any.tensor_tensor(
                    mb[:, :k, :],
                    dlt[:, :k, None].to_broadcast([P, k, R * P]),
                    iot[:, None, :].to_broadcast([P, k, R * P]),
                    op=mybir.AluOpType.is_equal,
                )
                for t in range(k):
                    g, first, last = consts["tiles2"][t_base + t]
                    xp = qxw.tile([P, R * O], f32, tag="xwp")
                    for r in range(R):
                        nc.tensor.matmul(
                            xp[:, r * O : (r + 1) * O],
                            lhsT=mst[:, 0, t * P : (t + 1) * P],
                            rhs=wts[r][:],
                            start=True,
                            stop=True,
                        )
                    xs = xpool.tile([P, R * O], f16, tag="xws")
                    nc.scalar.activation(xs[:], xp[:], AF.Copy, scale=wtt[:, t : t + 1])
                    if first:
                        psums[g] = qagg.tile([O, P], f32, tag="agg", name=f"agg2_{g}")
                    for r in range(R):
                        nc.tensor.matmul(
                            psums[g][:],
                            lhsT=xs[:, r * O : (r + 1) * O],
                            rhs=mb[:, t, r * P : (r + 1) * P],
                            start=first and r == 0,
                            stop=last and r == R - 1,
                        )
                    if last:
                        b = g % nblk
                        nc.any.tensor_add(
                            acc2[:, b * P : (b + 1) * P],
                            acc2[:, b * P : (b + 1) * P],
                            psums[g][:],
                        )
                        del psums[g]
                t_base += k

            # ---- out = (acc2 + b2)^T
            for b in range(nblk):
                sl = slice(b * P, (b + 1) * P)
                ob = epool.tile([O, P], f32, tag="ep_o")
                nc.any.tensor_add(ob[:], acc2[:, sl], b2t[:, 0:1].to_broadcast([O, P]))
                tp = qtr.tile([P, P], f32, tag="ep_t", name="tp_o")
                nc.tensor.transpose(out=tp[:, :O], in_=ob[:], identity=idt[:O, :O])
                of = epool.tile([P, O], f32, tag="ep_of")
                nc.scalar.activation(of[:], tp[:, :O], AF.Copy)
                nc.sync.dma_start(out=out[sl, :], in_=of[:])

    if finalize:
        nc.finalize()  # Bacc.compile(): register alloc + ISA codegen + lib loads
    return nc


last_exec_ns = None  # steady-state device wall time of the sharded executable


def _run_pjrt_timed(nc, in_maps, reps=4):
    """run_bass_via_pjrt with the sharded executable re-run and timed.

    Mirrors concourse.bass2jax.run_bass_via_pjrt's multi-core tail; outputs
    are donated zero buffers, so each rep gets fresh zeros. Steady-state
    wall time (min over reps 2..n, includes PJRT dispatch) goes to
    last_exec_ns.
    """
    import time

    import jax
    import jax.numpy as jnp
    from jax.experimental.shard_map import shard_map
    from jax.sharding import Mesh, PartitionSpec

    import concourse.mybir as mybir
    from concourse import bass2jax

    global last_exec_ns
    bass2jax.install_neuronx_cc_hook()
    n_cores = NCORES

    in_names, out_names, out_avals, zero_shapes = [], [], [], []
    for alloc in nc.m.functions[0].allocations:
        if not isinstance(alloc, mybir.MemoryLocationSet):
            continue
        name = alloc.memorylocations[0].name
        if alloc.kind == "ExternalInput":
            in_names.append(name)
        elif alloc.kind == "ExternalOutput":
            np_dt = mybir.dt.np(alloc.dtype)
            out_names.append(name)
            out_avals.append(jax.core.ShapedArray(tuple(alloc.tensor_shape), np_dt))
            zero_shapes.append((tuple(alloc.tensor_shape), np_dt))
    n_params, n_outs = len(in_names), len(out_names)
    all_in_names = tuple(in_names + out_names)

    def _body(*args):
        outs = bass2jax._bass_exec_p.bind(
            *args,
            out_avals=tuple(out_avals),
            in_names=all_in_names,
            out_names=tuple(out_names),
            lowering_input_output_aliases=(),
            sim_require_finite=True,
            sim_require_nnan=True,
            nc=nc,
        )
        return tuple(outs)

    devices = jax.devices()[:n_cores]
    mesh = Mesh(np.asarray(devices), ("core",))
    sharded = jax.jit(
        shard_map(
            _body,
            mesh=mesh,
            in_specs=(PartitionSpec("core"),) * (n_params + n_outs),
            out_specs=(PartitionSpec("core"),) * n_outs,
            check_rep=False,
        ),
        donate_argnums=tuple(range(n_params, n_params + n_outs)),
        keep_unused=True,
    )
    pid_name = nc.partition_id_tensor.name if nc.partition_id_tensor else None

    def _core_input(c, nm):
        if nm == pid_name:
            return np.array([[c]], dtype=np.uint32)
        return np.asarray(in_maps[c][nm])

    concat_in = [
        np.concatenate([_core_input(c, nm) for c in range(n_cores)], axis=0)
        for nm in in_names
    ]
    concat_in = [jax.device_put(a) for a in concat_in]

    def zeros():
        return [
            jnp.zeros((n_cores * s[0], *s[1:]), d) for (s, d) in zero_shapes
        ]

    times = []
    out_arrs = None
    for i in range(reps):
        z = zeros()
        jax.block_until_ready(z)
        t0 = time.perf_counter()
        out_arrs = sharded(*concat_in, *z)
        jax.block_until_ready(out_arrs)
        times.append(time.perf_counter() - t0)
    last_exec_ns = int(min(times[1:]) * 1e9)
    print(f"pjrt call times: {[f'{t * 1e3:.2f}ms' for t in times]}")
    return [
        np.asarray(out_arrs[i]).reshape(n_cores, *out_avals[i].shape)[c]
        for c in range(n_cores)
        for i in [0]
    ]


def kernel(embed, weight, bias1, bias2, edge_src, edge_dst):
    embed = np.asarray(embed)
    weight = np.asarray(weight)
    bias1 = np.asarray(bias1)
    bias2 = np.asarray(bias2)
    edge_src = np.asarray(edge_src)
    edge_dst = np.asarray(edge_dst)

    consts, in_maps = _host_schedules(embed, weight, bias1, bias2, edge_src, edge_dst)

    backend = os.environ.get("KERNEL_BACKEND", "hw")
    if backend == "numpy":
        return _simulate_numpy(consts, in_maps).astype(np.float32)

    nc = _build_program(consts, finalize=backend != "sim")

    if backend == "sim":
        from concourse.bass_interp import MultiCoreSim

        sim = MultiCoreSim(nc, NCORES)
        for k in range(NCORES):
            for name, arr in in_maps[k].items():
                sim.cores[k].tensor(name)[:] = arr
        sim.simulate()
        outs = [np.array(sim.cores[k].tensor("out")) for k in range(NCORES)]
    elif os.environ.get("KERNEL_TRACE", "0") == "1":
        outs = _run_pjrt_timed(nc, in_maps)
    else:
        from concourse.bass_utils import run_bass_kernel_spmd

        res = run_bass_kernel_spmd(nc, in_maps, list(range(NCORES)))
        global last_results
        last_results = res
        outs = [res.results[k]["out"] for k in range(NCORES)]

    full = np.concatenate(outs, 0)[: consts["N"]]
    return np.asarray(full, np.float32)

